# revision 37
# baseline (speedup 1.0000x reference)
"""Causal self-attention (B=2, T=2048, C=1024, H=16, D=64) on 8 trn2 NeuronCores.

Sharding: core i handles batch b = i//4 and heads [4*(i%4), 4*(i%4)+4).
Each core computes QKV projection for its head subset, causal attention, and
its partial output projection. Host sums the 4 per-batch partials (disjoint
head subsets -> the "all-reduce after proj" is a host-side sum) and adds bias.

v4 layout/schedule changes over v2:
  - DMA front is half0-first: all 8 x-chunk token-halves [0:1024) stream
    before any half1 bytes, so the ng=0 QK chains and V blocks t<8 are fed
    sooner.  ScalarE (a hwdge engine that also runs the exp stream) only
    issues the small early DMA set; all late-deadline bulk (x half1, wp)
    rides Sync so Scalar's DGE ring is drained before attention starts.
  - One flat software-pipelined slot stream across all (hp, qb) streams:
    S(i+1) always issues before PV(i), including across head-pair/q-block
    boundaries, and filler units (QK chains, V blocks, deferred proj
    sub-blocks) are injected per-slot to keep PE dense where ScalarE's exp
    stream is the local pacer (late q-blocks).
  - QK chains are single-psum-bank 8-matmul accumulation chains (V-block
    structure) instead of bank-alternating pairs.
  - The causal mask is applied in-place on the exp output by one gpsimd
    affine_select per diagonal slot (predicate i - p >= 0 with a [[0,2],
    [1,w]] pattern covering both heads) - no mask tile, no DVE multiplies.
  - Output projection is weight-stationary: wp [128,128] column slices as
    stationaries, psum out y^T[f, q], accumulating same-bank chains.
    y is written transposed (yT) and the host untransposes.  Proj for qb is
    decoupled from the norm flush and deferred into later slot streams
    where PE would otherwise idle.
  - Tail: the last block's normalization runs in two 256-column chunks,
    each immediately followed by its proj f-slice units (y DMAs on Scalar,
    reciprocal scatter/gathers on Sync).
"""

import numpy as np
import ml_dtypes
from contextlib import ExitStack

B, T, C, H, D = 2, 2048, 1024, 16, 64
NCORES = 8
HEADS_PER_CORE = 4  # 2 head-pairs
CCHUNKS = C // 128  # 8
TBLOCKS = T // 128  # 16
QBLOCKS = T // 512  # 4
NSLICES = T // 512  # 4 token n-slices

_CACHE = {}


def _build():
    import concourse.mybir as mybir
    import concourse.tile as tile
    from concourse import bacc

    F32 = mybir.dt.float32
    BF16 = mybir.dt.bfloat16
    EXPF = mybir.ActivationFunctionType.Exp

    nc = bacc.Bacc("TRN2", target_bir_lowering=False, debug=False,
                   num_devices=NCORES)

    xT = nc.dram_tensor("xT", (C, T), BF16, kind="ExternalInput")
    wqk = nc.dram_tensor("wqk", (C, 512), BF16, kind="ExternalInput")
    wv = nc.dram_tensor("wv", (C, 256), BF16, kind="ExternalInput")
    wp = nc.dram_tensor("wp", (256, C), BF16, kind="ExternalInput")
    yT = nc.dram_tensor("yT", (C, T), BF16, kind="ExternalOutput")

    with ExitStack() as ctx:
        tc = ctx.enter_context(tile.TileContext(nc))
        const = ctx.enter_context(tc.tile_pool(name="const", bufs=1))
        xw = ctx.enter_context(tc.tile_pool(name="xw", bufs=1))
        qkv = ctx.enter_context(tc.tile_pool(name="qkv", bufs=1))
        ppool = ctx.enter_context(tc.tile_pool(name="ppool", bufs=4))
        misc = ctx.enter_context(tc.tile_pool(name="misc", bufs=2))
        # PSUM budget (8 banks): psMM 4 (2 held by oaug pair) + psS 2*2
        psMM = ctx.enter_context(tc.tile_pool(name="psMM", bufs=4, space="PSUM"))
        psS = ctx.enter_context(tc.tile_pool(name="psS", bufs=2, space="PSUM"))

        # PE warmup on a dependency-free zero tile: keeps the HAM activity
        # monitor busy through the DMA front so real matmuls start at 2.4GHz
        # slimmer warmup: ~3.6us of cold N=256 matmuls flips HAM to full
        # clock right as the first data-fed chains arrive
        warm = const.tile([128, 384], BF16, name="warm", tag="warm")
        nc.vector.memset(warm, 0.0)
        warmps = psS.tile([128, 2, 512], F32, name="s", tag="s")
        for i in range(9):
            nc.tensor.matmul(warmps[:, 0, 0:256], warm[:, 0:128],
                             warm[:, 128:384], skip_group_check=True)
        # consts for the matmul-based tail normalization: a [1,1] ones
        # (rank-1 row->column transpose), a [128,64] ones (reciprocal
        # spread), and a [128,128] identity (column->replicated-rows)
        ones11 = const.tile([128, 1], F32, name="ones11", tag="ones11")
        nc.vector.memset(ones11, 1.0)
        ones64 = const.tile([128, 64], BF16, name="ones64", tag="ones64")
        nc.vector.memset(ones64, 1.0)
        ident = const.tile([128, 128], BF16, name="ident", tag="ident")
        nc.vector.memset(ident, 1.0)
        nc.gpsimd.affine_select(
            out=ident, in_=ident, compare_op=mybir.AluOpType.is_equal,
            fill=0.0, base=0, channel_multiplier=-1, pattern=[[1, 128]])

        # causal mask master: mask[p, i] = 1 if (i - 384 - p) >= 0 else 0;
        # slice [384-128j+n_off : 896-128j] is the diag-offset-j tile mask
        mask = const.tile([128, 896], BF16, name="mask", tag="mask")
        nc.vector.memset(mask, 1.0)
        nc.gpsimd.affine_select(
            out=mask, in_=mask, compare_op=mybir.AluOpType.is_ge,
            fill=0.0, base=-384, channel_multiplier=-1, pattern=[[1, 896]])

        # ---- input DMAs ----
        # Scalar is a hwdge engine but also runs the exp stream: give it
        # only the small early set (odd half0 chunks + weights) so its DMA
        # ring drains before attention starts; everything late-deadline
        # (all of half1, wp) rides Sync, whose next duties (norm
        # scatter/gathers) only begin mid-kernel.
        dma_engs = [nc.sync, nc.scalar]
        xc = [None] * CCHUNKS
        wqk_t = [None] * CCHUNKS
        wv_t = [None] * CCHUNKS
        # quarter q0 (token cols 0:512) feeds the n0 QK chains; q1 follows
        for c in range(CCHUNKS):
            t_ = xw.tile([128, T], BF16, name=f"x{c}", tag=f"x{c}")
            xc[c] = t_
            eng = dma_engs[c % 2]
            weng = dma_engs[(c + 1) % 2]
            eng.dma_start(out=t_[:, 0:512], in_=xT[c * 128:(c + 1) * 128, 0:512])
            w_ = xw.tile([128, 512], BF16, name=f"wqk{c}", tag=f"wqk{c}")
            weng.dma_start(out=w_, in_=wqk[c * 128:(c + 1) * 128, :])
            wqk_t[c] = w_
        # wv chunks (V blocks t<8 run in the front streams), then quarter q1
        for c in range(CCHUNKS):
            t_ = xw.tile([128, 256], BF16, name=f"wv{c}", tag=f"wv{c}")
            dma_engs[c % 2].dma_start(out=t_, in_=wv[c * 128:(c + 1) * 128, :])
            wv_t[c] = t_
        for c in range(CCHUNKS):
            dma_engs[c % 2].dma_start(
                out=xc[c][:, 512:1024], in_=xT[c * 128:(c + 1) * 128, 512:1024])
        # x token-half1 + wp: Sync only
        for c in range(CCHUNKS):
            nc.sync.dma_start(
                out=xc[c][:, 1024:2048], in_=xT[c * 128:(c + 1) * 128, 1024:2048])
        wp_t = []
        for ch in range(2):
            t_ = qkv.tile([128, C], BF16, name=f"wp{ch}", tag=f"wp{ch}")
            nc.sync.dma_start(out=t_, in_=wp[ch * 128:(ch + 1) * 128, :])
            wp_t.append(t_)

        # persistent QKV activation tiles
        qT = [qkv.tile([128, T], BF16, name=f"qT{i}", tag=f"qT{i}") for i in range(2)]
        kT = [qkv.tile([128, T], BF16, name=f"kT{i}", tag=f"kT{i}") for i in range(2)]
        vaug = [qkv.tile([128, HEADS_PER_CORE, D + 1], BF16, name=f"va{t}", tag=f"va{t}")
                for t in range(TBLOCKS)]
        # ones column of each V-augmented tile (softmax denominator source)
        for t in range(TBLOCKS):
            nc.gpsimd.memset(vaug[t][:, :, D], 1.0)
        opair = [qkv.tile([128, T], BF16, name=f"op{i}", tag=f"op{i}") for i in range(2)]

        def qk_unit(m, n, dst):
            """dst[:, n*512:(n+1)*512] = (wqk m-block).T @ x^T n-slice.

            Single-bank 8-matmul accumulation chain with a contiguous
            [128,128] stationary per chunk (V-block-like structure: the
            per-chunk LDWEIGHTS stream pipelines under the matmuls)."""
            ps = psMM.tile([128, 512], F32, name="mm", tag="mm")
            for c in range(CCHUNKS):
                nc.tensor.matmul(ps, wqk_t[c][:, m * 128:(m + 1) * 128],
                                 xc[c][:, n * 512:(n + 1) * 512],
                                 start=(c == 0), stop=(c == CCHUNKS - 1))
            nc.vector.tensor_copy(out=dst[:, n * 512:(n + 1) * 512], in_=ps)

        def v_tblock(t):
            """V for tokens [t*128, (t+1)*128) -> vaug[t][:, :, 0:64]"""
            ps = psMM.tile([128, 256], F32, name="mm", tag="mm")
            for c in range(CCHUNKS):
                nc.tensor.matmul(ps, xc[c][:, t * 128:(t + 1) * 128], wv_t[c],
                                 start=(c == 0), stop=(c == CCHUNKS - 1))
            # drain on ScalarE - its exp stream has mid-kernel slack and
            # this relieves the DVE, which carries casts, masks and norms
            nc.scalar.copy(
                out=vaug[t][:, :, 0:D],
                in_=ps.rearrange("p (h d) -> p h d", h=HEADS_PER_CORE))

        def proj_unit(qb, fs, c0=0, cw=512, yeng=None, scalar_drain=False):
            """yT rows [fs*128,(fs+1)*128), q cols [qb*512+c0, +cw): one
            weight-stationary same-bank chain over the two d-chunks."""
            q0 = qb * 512 + c0
            ps = psMM.tile([128, 512], F32, name="mm", tag="mm")
            for ch in range(2):
                nc.tensor.matmul(ps[:, 0:cw], wp_t[ch][:, fs * 128:(fs + 1) * 128],
                                 opair[ch][:, q0:q0 + cw],
                                 start=(ch == 0), stop=(ch == 1))
            yt = misc.tile([128, 512], BF16, name="yt", tag="yt", bufs=6)
            if scalar_drain:
                # tail-only: ScalarE's exp stream is finished, so its copy
                # engine parallels the DVE's norm multiplies
                nc.scalar.copy(out=yt[:, 0:cw], in_=ps[:, 0:cw])
            else:
                nc.vector.tensor_copy(out=yt[:, 0:cw], in_=ps[:, 0:cw])
            (yeng or nc.sync).dma_start(
                out=yT[fs * 128:(fs + 1) * 128, q0:q0 + cw], in_=yt[:, 0:cw])

        pending = []
        oaug_map = {}

        def emit_S(hp, qb, kb):
            """S^T pair + exp + causal mask for one key-block slot."""
            j = kb - 4 * qb  # >= 0 on diagonal band
            diag = j >= 0
            n_off = 128 * j if diag else 0
            sp = psS.tile([128, 2, 512], F32, name="s", tag="s")
            for h in range(2):
                nc.tensor.matmul(
                    sp[:, h, n_off:512],
                    kT[hp][64 * h:64 * h + 64, kb * 128:(kb + 1) * 128],
                    qT[hp][64 * h:64 * h + 64, qb * 512 + n_off:(qb + 1) * 512])
            pt = ppool.tile([128, 2, 512], BF16, name="p", tag="p")
            nc.scalar.activation(out=pt[:, :, n_off:512],
                                 in_=sp[:, :, n_off:512],
                                 func=EXPF, scale=1.0 / np.sqrt(D))
            if diag:
                # causal mask on DVE: GpSimd's queue holds the norm
                # broadcasts, which wait on slow gather DMAs and would
                # head-of-line block an affine_select placed there
                msl = mask[:, 384 - 128 * j + n_off:896 - 128 * j]
                for h in range(2):
                    nc.vector.tensor_mul(
                        pt[:, h, n_off:512], pt[:, h, n_off:512], msl)
            return (hp, qb, kb, pt, n_off)

        def emit_PV(slot):
            """PV accumulation; on the last key block drain O_aug to SBUF
            (frees both banks) and queue the deferred normalization."""
            hp, qb, kb, pt, n_off = slot
            oaug = oaug_map[(hp, qb)]
            last_kb = 4 * qb + 3
            for h in range(2):
                nc.tensor.matmul(
                    oaug[h][:, n_off:512],
                    vaug[kb][:, 2 * hp + h, :],
                    pt[:, h, n_off:512],
                    start=(kb == 0), stop=(kb == last_kb))
            if kb == last_kb:
                ous = []
                tail = (hp, qb) == (1, QBLOCKS - 1)
                for h in range(2):
                    ou = misc.tile([D + 1, 512], F32, name=f"ou{hp}{h}",
                                   tag=f"ou{hp}{h}", bufs=2)
                    if tail:
                        # drain the denominator row first so the tail's
                        # transpose matmuls start ~1us sooner
                        nc.vector.tensor_copy(out=ou[D:D + 1, :],
                                              in_=oaug[h][D:D + 1, :])
                        nc.vector.tensor_copy(out=ou[0:D, :],
                                              in_=oaug[h][0:D, :])
                    else:
                        nc.vector.tensor_copy(out=ou, in_=oaug[h])
                    ous.append(ou)
                pending.append((hp, qb, ous))

        def finish_norm(hp, qb, ous):
            """Softmax normalization for one (hp, qb): reciprocal of the 512
            rowsums via DMA-scatter across partitions (off critical path),
            then broadcast and divide.  Proj is scheduled separately."""
            for h in range(2):
                ou = ous[h]
                rb = misc.tile([128, 4], F32, name="rb", tag="rb")
                nc.sync.dma_start(
                    out=rb.unsqueeze(1),
                    in_=ou[D:D + 1, :].rearrange("p (a b) -> p a b", a=128))
                rbi = misc.tile([128, 4], F32, name="rbi", tag="rbi")
                nc.vector.reciprocal(out=rbi, in_=rb)
                r_inv = misc.tile([1, 512], F32, name="rinv", tag="rinv")
                nc.sync.dma_start(
                    out=r_inv.rearrange("p (a b) -> p a b", a=128),
                    in_=rbi.unsqueeze(1))
                r_rep = misc.tile([64, 512], F32, name="rrep", tag="rrep", bufs=2)
                nc.gpsimd.partition_broadcast(r_rep, r_inv, channels=64)
                if h == 0:
                    nc.vector.tensor_mul(
                        opair[hp][0:64, qb * 512:(qb + 1) * 512],
                        ou[0:D, :], r_rep)
                else:
                    # otmp hop rides Sync: a dma_start on Scalar would block
                    # the exp stream behind this whole chain's latency
                    otmp = misc.tile([64, 512], BF16, name="otmp", tag="otmp",
                                     bufs=1)
                    nc.vector.tensor_mul(otmp, ou[0:D, :], r_rep)
                    nc.sync.dma_start(
                        out=opair[hp][64:128, qb * 512:(qb + 1) * 512],
                        in_=otmp)

        def finish_norm_tail(hp, qb, ous):
            """Last-block normalization with zero DMA hops: rank-1 matmuls
            transpose the denominator rows into psum columns, DVE takes the
            reciprocals wide (4 per lane), tensor_scalar spreads each column
            64-wide, and identity matmuls replicate each reciprocal row
            across 64 psum partitions.  PE stays warm throughout, so the
            proj that follows runs at full clock."""
            # [128, 8] psum: column 4h+j holds denom q-slice j of head h
            tp = psMM.tile([128, 8], F32, name="mm", tag="mm")
            for h in range(2):
                for j in range(4):
                    nc.tensor.matmul(
                        tp[:, 4 * h + j:4 * h + j + 1],
                        ous[h][D:D + 1, 128 * j:128 * (j + 1)],
                        ones11[D:D + 1, :])
            rbi = misc.tile([128, 8], F32, name="rbi8", tag="rbi8")
            nc.vector.reciprocal(out=rbi, in_=tp)
            rreps = []
            for h in range(2):
                r_rep = psMM.tile([64, 512], F32, name="mm", tag="mm")
                for j in range(4):
                    rv = misc.tile([128, 64], BF16, name="rv", tag="rv", bufs=4)
                    nc.vector.tensor_scalar_mul(
                        rv, ones64, rbi[:, 4 * h + j:4 * h + j + 1])
                    nc.tensor.matmul(r_rep[:, 128 * j:128 * (j + 1)],
                                     rv, ident)
                rreps.append(r_rep)
            for h in range(2):
                if h == 0:
                    nc.vector.tensor_mul(
                        opair[hp][0:64, qb * 512:(qb + 1) * 512],
                        ous[h][0:D, :], rreps[h])
                else:
                    otmp = misc.tile([64, 512], BF16, name="otmp",
                                     tag="otmpt", bufs=2)
                    nc.vector.tensor_mul(otmp, ous[h][0:D, :], rreps[h])
                    nc.sync.dma_start(
                        out=opair[hp][64:128, qb * 512:(qb + 1) * 512],
                        in_=otmp)
            for fs in range(8):
                proj_unit(qb, fs, yeng=(nc.scalar if fs % 2 else nc.sync),
                          scalar_drain=(fs % 2 == 0))

        # ---- flat software-pipelined slot stream ----
        # slots in (qb, hp, kb) order; inj[(hp, qb, kb)] = filler units
        # emitted right after that slot's S, keeping PE dense through the
        # exp-paced stretches and meeting each unit's data deadline.
        slots = []
        for qb in range(QBLOCKS):
            for hp in range(2):
                for kb in range(4 * qb + 4):
                    slots.append((hp, qb, kb))

        inj = {}

        def add_inj(hp, qb, kb, fn):
            inj.setdefault((hp, qb, kb), []).append(fn)

        # (injections sit at kb >= 1 only: at kb==0 the previous stream's
        # oaug psum pair is still live, so a new psMM allocation there
        # would stall in the PE FIFO ahead of the pending PV)
        # hp0-qb0: V blocks 0..3 + the QK chains hp1-qb0 needs
        add_inj(0, 0, 1, lambda: qk_unit(1, 0, qT[1]))
        add_inj(0, 0, 1, lambda: v_tblock(0))
        add_inj(0, 0, 2, lambda: qk_unit(3, 0, kT[1]))
        add_inj(0, 0, 2, lambda: v_tblock(1))
        add_inj(0, 0, 3, lambda: v_tblock(2))
        add_inj(0, 0, 3, lambda: v_tblock(3))
        # hp1-qb0: V 4..7 + n1 slices of qT0/kT0 (for qb1)
        add_inj(1, 0, 1, lambda: qk_unit(0, 1, qT[0]))
        add_inj(1, 0, 1, lambda: v_tblock(4))
        add_inj(1, 0, 2, lambda: qk_unit(2, 1, kT[0]))
        add_inj(1, 0, 2, lambda: v_tblock(5))
        add_inj(1, 0, 3, lambda: v_tblock(6))
        add_inj(1, 0, 3, lambda: v_tblock(7))
        # hp0-qb1: n1 slices for hp1-qb1
        add_inj(0, 1, 1, lambda: qk_unit(1, 1, qT[1]))
        add_inj(0, 1, 3, lambda: qk_unit(3, 1, kT[1]))
        # hp1-qb1: n2 slices (for qb2)
        add_inj(1, 1, 1, lambda: qk_unit(0, 2, qT[0]))
        add_inj(1, 1, 3, lambda: qk_unit(2, 2, kT[0]))
        add_inj(1, 1, 5, lambda: qk_unit(1, 2, qT[1]))
        add_inj(1, 1, 7, lambda: qk_unit(3, 2, kT[1]))
        # hp0-qb2: V 8..11, n3 slices of qT0/kT0, proj(qb0) first half
        add_inj(0, 2, 1, lambda: v_tblock(8))
        add_inj(0, 2, 2, lambda: v_tblock(9))
        add_inj(0, 2, 3, lambda: v_tblock(10))
        add_inj(0, 2, 4, lambda: v_tblock(11))
        add_inj(0, 2, 5, lambda: qk_unit(0, 3, qT[0]))
        add_inj(0, 2, 7, lambda: qk_unit(2, 3, kT[0]))
        for i in range(4):
            add_inj(0, 2, 8 + i, lambda fs=i: proj_unit(0, fs))
        # hp1-qb2: n3 slices of qT1/kT1, rest of proj(qb0), proj(qb1) start
        add_inj(1, 2, 1, lambda: qk_unit(1, 3, qT[1]))
        add_inj(1, 2, 3, lambda: qk_unit(3, 3, kT[1]))
        for i in range(4):
            add_inj(1, 2, 4 + i, lambda fs=4 + i: proj_unit(0, fs))
        add_inj(1, 2, 9, lambda: proj_unit(1, 0))
        add_inj(1, 2, 11, lambda: proj_unit(1, 1))
        # hp0-qb3: V 12..15 (diag deadline kb=12), proj(qb1)/(qb2) fill
        add_inj(0, 3, 1, lambda: v_tblock(12))
        add_inj(0, 3, 2, lambda: v_tblock(13))
        add_inj(0, 3, 3, lambda: v_tblock(14))
        add_inj(0, 3, 4, lambda: v_tblock(15))
        for i in range(4):
            add_inj(0, 3, 5 + 2 * i, lambda fs=2 + i: proj_unit(1, fs))
        add_inj(0, 3, 13, lambda: proj_unit(2, 0))
        add_inj(0, 3, 15, lambda: proj_unit(2, 1))
        # hp1-qb3: remaining proj(qb1)/(qb2) keeps PE fed while exp paces
        add_inj(1, 3, 3, lambda: proj_unit(1, 6))
        add_inj(1, 3, 5, lambda: proj_unit(1, 7))
        for i, kb in enumerate([7, 9, 11, 13, 14, 15]):
            add_inj(1, 3, kb, lambda fs=2 + i: proj_unit(2, fs))

        # phase A: the two chains hp0-qb0 needs, fed by the half0 DMA sweep
        qk_unit(2, 0, kT[0])
        qk_unit(0, 0, qT[0])

        # iteration order per slot: norm flush, S(i), filler units, PV(i-1).
        # Fillers precede the PV so a PV waiting on exp/mask latency never
        # blocks ready matmuls in the PE FIFO.  The stream's oaug pair is
        # allocated at kb==1 (just before its first PV), after the previous
        # stream's pair was freed by its last PV's drain at kb==0.
        prev = None
        for (hp, qb, kb) in slots:
            if kb == 2:
                while pending:
                    finish_norm(*pending.pop(0))
            cur = emit_S(hp, qb, kb)
            for fn in inj.get((hp, qb, kb), []):
                fn()
            if kb == 1:
                oaug_map[(hp, qb)] = [
                    psMM.tile([D + 1, 512], F32, name="mm", tag="mm")
                    for h in range(2)]
            if prev is not None:
                emit_PV(prev)
            prev = cur
        emit_PV(prev)
        while pending:
            finish_norm_tail(*pending.pop(0))

    nc.compile()
    return nc


def _get_nc():
    if "nc" not in _CACHE:
        _CACHE["nc"] = _build()
    return _CACHE["nc"]


def _make_in_maps(inputs):
    x = np.asarray(inputs["x"], dtype=np.float32)
    Wqkv = np.asarray(inputs["Wqkv"], dtype=np.float32)
    Wproj = np.asarray(inputs["Wproj"], dtype=np.float32)
    in_maps = []
    for i in range(NCORES):
        b = i // 4
        g = i % 4
        f0 = g * 256  # first feature column of this core's 4 heads
        bf16 = ml_dtypes.bfloat16
        in_maps.append({
            "xT": np.ascontiguousarray(x[b].T.astype(bf16)),
            "wqk": np.ascontiguousarray(
                np.concatenate([Wqkv[:, f0:f0 + 256],
                                Wqkv[:, C + f0:C + f0 + 256]], axis=1).astype(bf16)),
            "wv": np.ascontiguousarray(
                Wqkv[:, 2 * C + f0:2 * C + f0 + 256].astype(bf16)),
            "wp": np.ascontiguousarray(Wproj[f0:f0 + 256, :].astype(bf16)),
        })
    return in_maps


def kernel(x, Wqkv, bqkv, Wproj, bproj):
    from concourse.bass_utils import run_bass_kernel_spmd

    bproj = np.asarray(bproj, dtype=np.float32)
    nc = _get_nc()
    in_maps = _make_in_maps({"x": x, "Wqkv": Wqkv, "Wproj": Wproj})

    res = run_bass_kernel_spmd(nc, in_maps, core_ids=list(range(NCORES)))

    out = np.zeros((B, T, C), dtype=np.float64)
    for i in range(NCORES):
        out[i // 4] += res.results[i]["yT"].astype(np.float64).T
    out += bproj.astype(np.float64)
    return out.astype(np.float32)


# revision 38
# speedup vs baseline: 1.0192x; 1.0192x over previous
"""Causal self-attention (B=2, T=2048, C=1024, H=16, D=64) on 8 trn2 NeuronCores.

Sharding: core i handles batch b = i//4 and heads [4*(i%4), 4*(i%4)+4).
Each core computes QKV projection for its head subset, causal attention, and
its partial output projection. Host sums the 4 per-batch partials (disjoint
head subsets -> the "all-reduce after proj" is a host-side sum) and adds bias.

Schedule (v9; 183.6us baseline -> ~154us):
  - DMA front is quarter-q0-first: token cols [0:512) of every x chunk and
    the wqk tiles stream before anything else, feeding the first QK chains
    at ~9us.  ScalarE (a hwdge engine that also runs the exp stream) only
    issues the small early DMA set; all late-deadline bulk (x half1, wp)
    rides Sync so Scalar's DGE ring is drained before attention starts -
    a dma_start on an engine blocks that engine's FIFO until ring space.
  - One flat software-pipelined slot stream across all (hp, qb) streams:
    S(i+1) and filler units always issue before PV(i), including across
    head-pair/q-block boundaries, so a PV waiting on exp latency never
    holds ready matmuls back in the PE FIFO.  Filler units (QK chains, V
    blocks, deferred proj f-slices) are placed per-slot to keep PE dense
    where ScalarE's exp stream is the local pacer (late q-blocks).  Each
    stream's O-psum pair is allocated at kb==1, after the previous
    stream's pair was freed by its last PV drain.
  - QK chains are single-psum-bank 8-matmul accumulation chains.
  - The causal mask is a DVE multiply with a precomputed mask tile; it
    deliberately stays OFF GpSimd, whose queue holds the norm broadcasts
    that wait on slow 16B-segment gather DMAs (head-of-line blocking).
  - V-block drains run on ScalarE (mid-kernel slack) to relieve DVE.
  - Output projection is weight-stationary: wp [128,128] column slices as
    stationaries accumulate same-bank chains into psum y^T[f, q]; y is
    written transposed (yT) and the host untransposes.  Proj for qb is
    decoupled from the norm flush and deferred into the late exp-paced
    slot streams where PE would otherwise idle.
  - Non-tail softmax normalization: denominator row DMA-scattered across
    128 partitions (4/lane reciprocals), gathered back, gpsimd-broadcast,
    divided - all off the critical path, flushed at the next stream's
    kb==2 with the h1 copy-hop on Sync.
  - Tail (last block) normalization avoids DMA entirely: rank-1 matmuls
    transpose the denominator rows into psum columns, the reciprocal runs
    4-per-lane on DVE, tensor_scalar spreads each reciprocal column, and
    identity matmuls replicate it across 64 psum partitions; PE stays warm
    (no HAM re-throttle) and the final proj drains ride ScalarE.
"""

import numpy as np
import ml_dtypes
from contextlib import ExitStack

B, T, C, H, D = 2, 2048, 1024, 16, 64
NCORES = 8
HEADS_PER_CORE = 4  # 2 head-pairs
CCHUNKS = C // 128  # 8
TBLOCKS = T // 128  # 16
QBLOCKS = T // 512  # 4
NSLICES = T // 512  # 4 token n-slices

_CACHE = {}


def _build():
    import concourse.mybir as mybir
    import concourse.tile as tile
    from concourse import bacc

    F32 = mybir.dt.float32
    BF16 = mybir.dt.bfloat16
    EXPF = mybir.ActivationFunctionType.Exp

    nc = bacc.Bacc("TRN2", target_bir_lowering=False, debug=False,
                   num_devices=NCORES)

    xT = nc.dram_tensor("xT", (C, T), BF16, kind="ExternalInput")
    wqk = nc.dram_tensor("wqk", (C, 512), BF16, kind="ExternalInput")
    wv = nc.dram_tensor("wv", (C, 256), BF16, kind="ExternalInput")
    wp = nc.dram_tensor("wp", (256, C), BF16, kind="ExternalInput")
    yT = nc.dram_tensor("yT", (C, T), BF16, kind="ExternalOutput")

    with ExitStack() as ctx:
        tc = ctx.enter_context(tile.TileContext(nc))
        const = ctx.enter_context(tc.tile_pool(name="const", bufs=1))
        xw = ctx.enter_context(tc.tile_pool(name="xw", bufs=1))
        qkv = ctx.enter_context(tc.tile_pool(name="qkv", bufs=1))
        ppool = ctx.enter_context(tc.tile_pool(name="ppool", bufs=4))
        misc = ctx.enter_context(tc.tile_pool(name="misc", bufs=2))
        # PSUM budget (8 banks): psMM 4 (2 held by oaug pair) + psS 2*2
        psMM = ctx.enter_context(tc.tile_pool(name="psMM", bufs=4, space="PSUM"))
        psS = ctx.enter_context(tc.tile_pool(name="psS", bufs=2, space="PSUM"))

        # PE warmup on a dependency-free zero tile: keeps the HAM activity
        # monitor busy through the DMA front so real matmuls start at 2.4GHz
        # slimmer warmup: ~3.6us of cold N=256 matmuls flips HAM to full
        # clock right as the first data-fed chains arrive
        warm = const.tile([128, 384], BF16, name="warm", tag="warm")
        nc.vector.memset(warm, 0.0)
        warmps = psS.tile([128, 2, 512], F32, name="s", tag="s")
        for i in range(9):
            nc.tensor.matmul(warmps[:, 0, 0:256], warm[:, 0:128],
                             warm[:, 128:384], skip_group_check=True)
        # consts for the matmul-based tail normalization: a [1,1] ones
        # (rank-1 row->column transpose), a [128,64] ones (reciprocal
        # spread), and a [128,128] identity (column->replicated-rows)
        ones11 = const.tile([128, 1], F32, name="ones11", tag="ones11")
        nc.vector.memset(ones11, 1.0)
        ones64 = const.tile([128, 64], BF16, name="ones64", tag="ones64")
        nc.vector.memset(ones64, 1.0)
        ident = const.tile([128, 128], BF16, name="ident", tag="ident")
        nc.vector.memset(ident, 1.0)
        nc.gpsimd.affine_select(
            out=ident, in_=ident, compare_op=mybir.AluOpType.is_equal,
            fill=0.0, base=0, channel_multiplier=-1, pattern=[[1, 128]])

        # causal mask master: mask[p, i] = 1 if (i - 384 - p) >= 0 else 0;
        # slice [384-128j+n_off : 896-128j] is the diag-offset-j tile mask
        mask = const.tile([128, 896], BF16, name="mask", tag="mask")
        nc.vector.memset(mask, 1.0)
        nc.gpsimd.affine_select(
            out=mask, in_=mask, compare_op=mybir.AluOpType.is_ge,
            fill=0.0, base=-384, channel_multiplier=-1, pattern=[[1, 896]])

        # ---- input DMAs ----
        # Scalar is a hwdge engine but also runs the exp stream: give it
        # only the small early set (odd half0 chunks + weights) so its DMA
        # ring drains before attention starts; everything late-deadline
        # (all of half1, wp) rides Sync, whose next duties (norm
        # scatter/gathers) only begin mid-kernel.
        dma_engs = [nc.sync, nc.scalar]
        xc = [None] * CCHUNKS
        wqk_t = [None] * CCHUNKS
        wv_t = [None] * CCHUNKS
        # quarter q0 (token cols 0:512) feeds the n0 QK chains; q1 follows
        for c in range(CCHUNKS):
            t_ = xw.tile([128, T], BF16, name=f"x{c}", tag=f"x{c}")
            xc[c] = t_
            eng = dma_engs[c % 2]
            weng = dma_engs[(c + 1) % 2]
            eng.dma_start(out=t_[:, 0:512], in_=xT[c * 128:(c + 1) * 128, 0:512])
            w_ = xw.tile([128, 512], BF16, name=f"wqk{c}", tag=f"wqk{c}")
            weng.dma_start(out=w_, in_=wqk[c * 128:(c + 1) * 128, :])
            wqk_t[c] = w_
        # wv chunks (V blocks t<8 run in the front streams), then quarter q1
        for c in range(CCHUNKS):
            t_ = xw.tile([128, 256], BF16, name=f"wv{c}", tag=f"wv{c}")
            dma_engs[c % 2].dma_start(out=t_, in_=wv[c * 128:(c + 1) * 128, :])
            wv_t[c] = t_
        for c in range(CCHUNKS):
            dma_engs[c % 2].dma_start(
                out=xc[c][:, 512:1024], in_=xT[c * 128:(c + 1) * 128, 512:1024])
        # x token-half1 + wp: Sync only
        for c in range(CCHUNKS):
            nc.sync.dma_start(
                out=xc[c][:, 1024:2048], in_=xT[c * 128:(c + 1) * 128, 1024:2048])
        wp_t = []
        for ch in range(2):
            t_ = qkv.tile([128, C], BF16, name=f"wp{ch}", tag=f"wp{ch}")
            nc.sync.dma_start(out=t_, in_=wp[ch * 128:(ch + 1) * 128, :])
            wp_t.append(t_)

        # persistent QKV activation tiles
        qT = [qkv.tile([128, T], BF16, name=f"qT{i}", tag=f"qT{i}") for i in range(2)]
        kT = [qkv.tile([128, T], BF16, name=f"kT{i}", tag=f"kT{i}") for i in range(2)]
        vaug = [qkv.tile([128, HEADS_PER_CORE, D + 1], BF16, name=f"va{t}", tag=f"va{t}")
                for t in range(TBLOCKS)]
        # ones column of each V-augmented tile (softmax denominator source)
        for t in range(TBLOCKS):
            nc.gpsimd.memset(vaug[t][:, :, D], 1.0)
        opair = [qkv.tile([128, T], BF16, name=f"op{i}", tag=f"op{i}") for i in range(2)]

        def qk_unit(m, n, dst):
            """dst[:, n*512:(n+1)*512] = (wqk m-block).T @ x^T n-slice.

            Single-bank 8-matmul accumulation chain with a contiguous
            [128,128] stationary per chunk (V-block-like structure: the
            per-chunk LDWEIGHTS stream pipelines under the matmuls)."""
            ps = psMM.tile([128, 512], F32, name="mm", tag="mm")
            for c in range(CCHUNKS):
                nc.tensor.matmul(ps, wqk_t[c][:, m * 128:(m + 1) * 128],
                                 xc[c][:, n * 512:(n + 1) * 512],
                                 start=(c == 0), stop=(c == CCHUNKS - 1))
            nc.vector.tensor_copy(out=dst[:, n * 512:(n + 1) * 512], in_=ps)

        def v_tblock(t):
            """V for tokens [t*128, (t+1)*128) -> vaug[t][:, :, 0:64]"""
            ps = psMM.tile([128, 256], F32, name="mm", tag="mm")
            for c in range(CCHUNKS):
                nc.tensor.matmul(ps, xc[c][:, t * 128:(t + 1) * 128], wv_t[c],
                                 start=(c == 0), stop=(c == CCHUNKS - 1))
            # drain on ScalarE - its exp stream has mid-kernel slack and
            # this relieves the DVE, which carries casts, masks and norms
            nc.scalar.copy(
                out=vaug[t][:, :, 0:D],
                in_=ps.rearrange("p (h d) -> p h d", h=HEADS_PER_CORE))

        def proj_unit(qb, fs, c0=0, cw=512, yeng=None, scalar_drain=False):
            """yT rows [fs*128,(fs+1)*128), q cols [qb*512+c0, +cw): one
            weight-stationary same-bank chain over the two d-chunks."""
            q0 = qb * 512 + c0
            ps = psMM.tile([128, 512], F32, name="mm", tag="mm")
            for ch in range(2):
                nc.tensor.matmul(ps[:, 0:cw], wp_t[ch][:, fs * 128:(fs + 1) * 128],
                                 opair[ch][:, q0:q0 + cw],
                                 start=(ch == 0), stop=(ch == 1))
            yt = misc.tile([128, 512], BF16, name="yt", tag="yt", bufs=6)
            if scalar_drain:
                # tail-only: ScalarE's exp stream is finished, so its copy
                # engine parallels the DVE's norm multiplies
                nc.scalar.copy(out=yt[:, 0:cw], in_=ps[:, 0:cw])
            else:
                nc.vector.tensor_copy(out=yt[:, 0:cw], in_=ps[:, 0:cw])
            (yeng or nc.sync).dma_start(
                out=yT[fs * 128:(fs + 1) * 128, q0:q0 + cw], in_=yt[:, 0:cw])

        pending = []
        oaug_map = {}

        def emit_S(hp, qb, kb):
            """S^T pair + exp + causal mask for one key-block slot."""
            j = kb - 4 * qb  # >= 0 on diagonal band
            diag = j >= 0
            n_off = 128 * j if diag else 0
            sp = psS.tile([128, 2, 512], F32, name="s", tag="s")
            for h in range(2):
                nc.tensor.matmul(
                    sp[:, h, n_off:512],
                    kT[hp][64 * h:64 * h + 64, kb * 128:(kb + 1) * 128],
                    qT[hp][64 * h:64 * h + 64, qb * 512 + n_off:(qb + 1) * 512])
            pt = ppool.tile([128, 2, 512], BF16, name="p", tag="p")
            nc.scalar.activation(out=pt[:, :, n_off:512],
                                 in_=sp[:, :, n_off:512],
                                 func=EXPF, scale=1.0 / np.sqrt(D))
            if diag:
                # causal mask on DVE: GpSimd's queue holds the norm
                # broadcasts, which wait on slow gather DMAs and would
                # head-of-line block an affine_select placed there
                msl = mask[:, 384 - 128 * j + n_off:896 - 128 * j]
                for h in range(2):
                    nc.vector.tensor_mul(
                        pt[:, h, n_off:512], pt[:, h, n_off:512], msl)
            return (hp, qb, kb, pt, n_off)

        def emit_PV(slot):
            """PV accumulation; on the last key block drain O_aug to SBUF
            (frees both banks) and queue the deferred normalization."""
            hp, qb, kb, pt, n_off = slot
            oaug = oaug_map[(hp, qb)]
            last_kb = 4 * qb + 3
            for h in range(2):
                nc.tensor.matmul(
                    oaug[h][:, n_off:512],
                    vaug[kb][:, 2 * hp + h, :],
                    pt[:, h, n_off:512],
                    start=(kb == 0), stop=(kb == last_kb))
            if kb == last_kb:
                ous = []
                tail = (hp, qb) == (1, QBLOCKS - 1)
                for h in range(2):
                    ou = misc.tile([D + 1, 512], F32, name=f"ou{hp}{h}",
                                   tag=f"ou{hp}{h}", bufs=2)
                    if tail:
                        # drain the denominator row first so the tail's
                        # transpose matmuls start ~1us sooner
                        nc.vector.tensor_copy(out=ou[D:D + 1, :],
                                              in_=oaug[h][D:D + 1, :])
                        nc.vector.tensor_copy(out=ou[0:D, :],
                                              in_=oaug[h][0:D, :])
                    else:
                        nc.vector.tensor_copy(out=ou, in_=oaug[h])
                    ous.append(ou)
                pending.append((hp, qb, ous))

        def finish_norm(hp, qb, ous):
            """Softmax normalization for one (hp, qb): reciprocal of the 512
            rowsums via DMA-scatter across partitions (off critical path),
            then broadcast and divide.  Proj is scheduled separately."""
            for h in range(2):
                ou = ous[h]
                rb = misc.tile([128, 4], F32, name="rb", tag="rb")
                nc.sync.dma_start(
                    out=rb.unsqueeze(1),
                    in_=ou[D:D + 1, :].rearrange("p (a b) -> p a b", a=128))
                rbi = misc.tile([128, 4], F32, name="rbi", tag="rbi")
                nc.vector.reciprocal(out=rbi, in_=rb)
                r_inv = misc.tile([1, 512], F32, name="rinv", tag="rinv")
                nc.sync.dma_start(
                    out=r_inv.rearrange("p (a b) -> p a b", a=128),
                    in_=rbi.unsqueeze(1))
                r_rep = misc.tile([64, 512], F32, name="rrep", tag="rrep", bufs=2)
                nc.gpsimd.partition_broadcast(r_rep, r_inv, channels=64)
                if h == 0:
                    nc.vector.tensor_mul(
                        opair[hp][0:64, qb * 512:(qb + 1) * 512],
                        ou[0:D, :], r_rep)
                else:
                    # otmp hop rides Sync: a dma_start on Scalar would block
                    # the exp stream behind this whole chain's latency
                    otmp = misc.tile([64, 512], BF16, name="otmp", tag="otmp",
                                     bufs=1)
                    nc.vector.tensor_mul(otmp, ou[0:D, :], r_rep)
                    nc.sync.dma_start(
                        out=opair[hp][64:128, qb * 512:(qb + 1) * 512],
                        in_=otmp)

        def finish_norm_tail(hp, qb, ous):
            """Last-block normalization with zero DMA hops: rank-1 matmuls
            transpose the denominator rows into psum columns, DVE takes the
            reciprocals wide (4 per lane), tensor_scalar spreads each column
            64-wide, and identity matmuls replicate each reciprocal row
            across 64 psum partitions.  PE stays warm throughout, so the
            proj that follows runs at full clock."""
            # [128, 8] psum: column 4h+j holds denom q-slice j of head h
            tp = psMM.tile([128, 8], F32, name="mm", tag="mm")
            for h in range(2):
                for j in range(4):
                    nc.tensor.matmul(
                        tp[:, 4 * h + j:4 * h + j + 1],
                        ous[h][D:D + 1, 128 * j:128 * (j + 1)],
                        ones11[D:D + 1, :])
            rbi = misc.tile([128, 8], F32, name="rbi8", tag="rbi8")
            nc.vector.reciprocal(out=rbi, in_=tp)
            rreps = []
            for h in range(2):
                r_rep = psMM.tile([64, 512], F32, name="mm", tag="mm")
                for j in range(4):
                    rv = misc.tile([128, 64], BF16, name="rv", tag="rv", bufs=4)
                    nc.vector.tensor_scalar_mul(
                        rv, ones64, rbi[:, 4 * h + j:4 * h + j + 1])
                    nc.tensor.matmul(r_rep[:, 128 * j:128 * (j + 1)],
                                     rv, ident)
                rreps.append(r_rep)
            for h in range(2):
                if h == 0:
                    nc.vector.tensor_mul(
                        opair[hp][0:64, qb * 512:(qb + 1) * 512],
                        ous[h][0:D, :], rreps[h])
                else:
                    otmp = misc.tile([64, 512], BF16, name="otmp",
                                     tag="otmpt", bufs=2)
                    nc.vector.tensor_mul(otmp, ous[h][0:D, :], rreps[h])
                    nc.sync.dma_start(
                        out=opair[hp][64:128, qb * 512:(qb + 1) * 512],
                        in_=otmp)
            for fs in range(8):
                proj_unit(qb, fs, yeng=(nc.scalar if fs % 2 else nc.sync),
                          scalar_drain=(fs % 2 == 0))

        # ---- flat software-pipelined slot stream ----
        # slots in (qb, hp, kb) order; inj[(hp, qb, kb)] = filler units
        # emitted right after that slot's S, keeping PE dense through the
        # exp-paced stretches and meeting each unit's data deadline.
        slots = []
        for qb in range(QBLOCKS):
            for hp in range(2):
                for kb in range(4 * qb + 4):
                    slots.append((hp, qb, kb))

        inj = {}

        def add_inj(hp, qb, kb, fn):
            inj.setdefault((hp, qb, kb), []).append(fn)

        # (injections sit at kb >= 1 only: at kb==0 the previous stream's
        # oaug psum pair is still live, so a new psMM allocation there
        # would stall in the PE FIFO ahead of the pending PV)
        # hp0-qb0: V blocks 0..3 + the QK chains hp1-qb0 needs
        add_inj(0, 0, 1, lambda: qk_unit(1, 0, qT[1]))
        add_inj(0, 0, 1, lambda: v_tblock(0))
        add_inj(0, 0, 2, lambda: qk_unit(3, 0, kT[1]))
        add_inj(0, 0, 2, lambda: v_tblock(1))
        add_inj(0, 0, 3, lambda: v_tblock(2))
        add_inj(0, 0, 3, lambda: v_tblock(3))
        # hp1-qb0: V 4..7 + n1 slices of qT0/kT0 (for qb1)
        add_inj(1, 0, 1, lambda: qk_unit(0, 1, qT[0]))
        add_inj(1, 0, 1, lambda: v_tblock(4))
        add_inj(1, 0, 2, lambda: qk_unit(2, 1, kT[0]))
        add_inj(1, 0, 2, lambda: v_tblock(5))
        add_inj(1, 0, 3, lambda: v_tblock(6))
        add_inj(1, 0, 3, lambda: v_tblock(7))
        # hp0-qb1: n1 slices for hp1-qb1
        add_inj(0, 1, 1, lambda: qk_unit(1, 1, qT[1]))
        add_inj(0, 1, 3, lambda: qk_unit(3, 1, kT[1]))
        # hp1-qb1: n2 slices (for qb2)
        add_inj(1, 1, 1, lambda: qk_unit(0, 2, qT[0]))
        add_inj(1, 1, 3, lambda: qk_unit(2, 2, kT[0]))
        add_inj(1, 1, 5, lambda: qk_unit(1, 2, qT[1]))
        add_inj(1, 1, 7, lambda: qk_unit(3, 2, kT[1]))
        # hp0-qb2: V 8..11, n3 slices of qT0/kT0, proj(qb0) first half
        add_inj(0, 2, 1, lambda: v_tblock(8))
        add_inj(0, 2, 2, lambda: v_tblock(9))
        add_inj(0, 2, 3, lambda: v_tblock(10))
        add_inj(0, 2, 4, lambda: v_tblock(11))
        add_inj(0, 2, 5, lambda: qk_unit(0, 3, qT[0]))
        add_inj(0, 2, 7, lambda: qk_unit(2, 3, kT[0]))
        for i in range(4):
            add_inj(0, 2, 8 + i, lambda fs=i: proj_unit(0, fs))
        # hp1-qb2: n3 slices of qT1/kT1, rest of proj(qb0), proj(qb1) start
        add_inj(1, 2, 1, lambda: qk_unit(1, 3, qT[1]))
        add_inj(1, 2, 3, lambda: qk_unit(3, 3, kT[1]))
        for i in range(4):
            add_inj(1, 2, 4 + i, lambda fs=4 + i: proj_unit(0, fs))
        add_inj(1, 2, 9, lambda: proj_unit(1, 0))
        add_inj(1, 2, 11, lambda: proj_unit(1, 1))
        # hp0-qb3: V 12..15 (diag deadline kb=12), proj(qb1)/(qb2) fill
        add_inj(0, 3, 1, lambda: v_tblock(12))
        add_inj(0, 3, 2, lambda: v_tblock(13))
        add_inj(0, 3, 3, lambda: v_tblock(14))
        add_inj(0, 3, 4, lambda: v_tblock(15))
        for i in range(4):
            add_inj(0, 3, 5 + 2 * i, lambda fs=2 + i: proj_unit(1, fs))
        add_inj(0, 3, 13, lambda: proj_unit(2, 0))
        add_inj(0, 3, 15, lambda: proj_unit(2, 1))
        # hp1-qb3: remaining proj(qb1)/(qb2) keeps PE fed while exp paces
        add_inj(1, 3, 3, lambda: proj_unit(1, 6))
        add_inj(1, 3, 5, lambda: proj_unit(1, 7))
        for i, kb in enumerate([7, 9, 11, 13, 14, 15]):
            add_inj(1, 3, kb, lambda fs=2 + i: proj_unit(2, fs))

        # phase A: the two chains hp0-qb0 needs, fed by the half0 DMA sweep
        qk_unit(2, 0, kT[0])
        qk_unit(0, 0, qT[0])

        # iteration order per slot: norm flush, S(i), filler units, PV(i-1).
        # Fillers precede the PV so a PV waiting on exp/mask latency never
        # blocks ready matmuls in the PE FIFO.  The stream's oaug pair is
        # allocated at kb==1 (just before its first PV), after the previous
        # stream's pair was freed by its last PV's drain at kb==0.
        prev = None
        for (hp, qb, kb) in slots:
            if kb == 2:
                while pending:
                    finish_norm(*pending.pop(0))
            cur = emit_S(hp, qb, kb)
            for fn in inj.get((hp, qb, kb), []):
                fn()
            if kb == 1:
                oaug_map[(hp, qb)] = [
                    psMM.tile([D + 1, 512], F32, name="mm", tag="mm")
                    for h in range(2)]
            if prev is not None:
                emit_PV(prev)
            prev = cur
        emit_PV(prev)
        while pending:
            finish_norm_tail(*pending.pop(0))

    nc.compile()
    return nc


def _get_nc():
    if "nc" not in _CACHE:
        _CACHE["nc"] = _build()
    return _CACHE["nc"]


def _make_in_maps(inputs):
    x = np.asarray(inputs["x"], dtype=np.float32)
    Wqkv = np.asarray(inputs["Wqkv"], dtype=np.float32)
    Wproj = np.asarray(inputs["Wproj"], dtype=np.float32)
    in_maps = []
    for i in range(NCORES):
        b = i // 4
        g = i % 4
        f0 = g * 256  # first feature column of this core's 4 heads
        bf16 = ml_dtypes.bfloat16
        in_maps.append({
            "xT": np.ascontiguousarray(x[b].T.astype(bf16)),
            "wqk": np.ascontiguousarray(
                np.concatenate([Wqkv[:, f0:f0 + 256],
                                Wqkv[:, C + f0:C + f0 + 256]], axis=1).astype(bf16)),
            "wv": np.ascontiguousarray(
                Wqkv[:, 2 * C + f0:2 * C + f0 + 256].astype(bf16)),
            "wp": np.ascontiguousarray(Wproj[f0:f0 + 256, :].astype(bf16)),
        })
    return in_maps


def kernel(x, Wqkv, bqkv, Wproj, bproj):
    from concourse.bass_utils import run_bass_kernel_spmd

    bproj = np.asarray(bproj, dtype=np.float32)
    nc = _get_nc()
    in_maps = _make_in_maps({"x": x, "Wqkv": Wqkv, "Wproj": Wproj})

    res = run_bass_kernel_spmd(nc, in_maps, core_ids=list(range(NCORES)))

    out = np.zeros((B, T, C), dtype=np.float64)
    for i in range(NCORES):
        out[i // 4] += res.results[i]["yT"].astype(np.float64).T
    out += bproj.astype(np.float64)
    return out.astype(np.float32)
